# revision 27
# baseline (speedup 1.0000x reference)
"""MoE (top-2 of 8 experts) Trainium2 kernel.

Strategy (load-balanced expert-parallel over 8 NeuronCores):
  - Router (x @ Wr -> softmax -> top-2 -> renormalize) runs on host: it is
    ~0.1% of total FLOPs and produces the token->expert dispatch that defines
    the sharding itself.
  - Token slots per core: four "A" tiles (512+512+512+444 = 1980 tokens of
    the core's primary expert) plus one "B" tile (128 tokens of a secondary
    expert). W1B/W3B preload into spare SBUF; W2B (packed m-major) reloads
    into W2A's SBUF mid-kernel with quarter-DMAs that chase the last A
    tile's layer-2 consumption. Heavy experts' overflow beyond their own
    core's slots is absorbed by lightly-loaded cores' B slots, cutting
    per-core capacity from the max expert load (2182) to 2108.
  - Each tile runs the 3-layer MLP in a transposed dataflow:
        h1T = relu(W1^T x^T + b1)   [H,  C]
        h2T = relu(W2^T h1T + b2)   [H2, C]
        yT  = W3^T h2T + b3         [O,  C]
    All matmul contractions sit on the partition axis, so no on-chip
    transposes are needed anywhere.
  - Layer 3's eight [128,10] matmuls are packed 4-at-a-time into the PE's
    column groups (tile_position=(0,32j)), cutting its streaming cost 4x;
    the four 10-row strips are reduced on the DVE into one staging buffer
    that is shipped once at the end (SWDGE, one sync wait).
  - Input DMAs are ordered so the first matmul's operands (w1 group 0, x
    tile 0) land first; dummy warm-up matmuls keep the PE's HAM clock gate
    at full rate through the initial DMA window.
  - Matmuls run in bf16 with fp32 PSUM accumulation (measured ~4e-3 max
    relative error vs the fp32 reference).
  - Any tokens that don't fit the static slot structure (never observed for
    the benchmark routing) are processed in extra rounds of a small
    overflow NEFF - correctness never depends on the slot capacities.
"""

import re as _re

import numpy as np
import ml_dtypes

import bass_rust as _bass_rust
import concourse.bass as bass
import concourse.mybir as mybir
import concourse.tile as tile
from concourse.bass_utils import run_bass_kernel_spmd


def _split_drain_and_barrier(self, tick_clock, wait_clock):
    """Replacement for TileContext._drain_and_barrier.

    The stock version hangs every outstanding proc semaphore wait on one
    Drain instruction; the walrus in this environment rejects any
    instruction carrying more than one sync wait. Emit the same waits as
    individual sync-engine wait_ge instructions (one wait each) before a
    clean drain instead.
    """
    ticks = [
        int(v)
        for v in _re.findall(r"\d+", repr(tick_clock.global_clock))
    ]
    for proc, sem in sorted(self.sems.allocated().items()):
        if proc < len(ticks) and ticks[proc] > 0:
            self.nc.sync.wait_ge(sem, _bass_rust.tick_to_sem(ticks[proc], proc))
    self.nc.sync.drain()

    self.nc.all_engine_barrier()
    assert self.sems is not None
    popped = self.nc._tile_sem_poison_stack.pop()
    assert popped is self._sem_poison
    self.nc.clear_and_free_semaphores(list(self.sems.allocated().values()))
    self.nc.all_engine_barrier()


tile.TileContext._drain_and_barrier = _split_drain_and_barrier

B, D, H, E, O, TOP_K = 8192, 1024, 2048, 8, 10, 2
H2 = H // 2
NCORES = 8
P = 128

# A tiles hold the core's primary expert; the B tile holds a (possibly
# different) secondary expert whose weights are reloaded mid-kernel.
A_TWS = [512, 512, 512, 444]
B_W = 122
A_CAP = sum(A_TWS)            # 1980
C = A_CAP + B_W               # 2108 per-core slots
OVERFLOW_TWS = [512]

KD = D // P       # 8   k-chunks for layer 1
MH = H // P       # 16  m-tiles for layer 1 / k-chunks for layer 2
MH2 = H2 // P     # 8   m-tiles for layer 2 / k-chunks for layer 3
NSTRIP = 4        # layer-3 column groups (output strips at partition 32j)

BF16 = mybir.dt.bfloat16
F32 = mybir.dt.float32
_nbf16 = ml_dtypes.bfloat16


NW1 = KD * H          # w1 columns in the packed weight tile
NW2 = MH * H2         # w2 columns
NW3 = MH2 * O         # w3 columns
# w1 arrives m-major in groups: small leading groups so the m-loop starts
# as soon as possible, then full-rate groups that stay ahead of compute.
W1GROUPS = [2, 2, 4, 4, 4]
assert sum(W1GROUPS) == MH
N_WARM = 10           # PE warm-up matmuls issued during the DMA head


def _build_nc(with_bias: bool, tws, b_width: int | None) -> bass.Bass:
    cap = sum(tws) + (b_width or 0)
    nc = bass.Bass()
    # Host pre-packs everything into the on-chip layout:
    #  xt   [128, KD, C]  — x gathered/transposed, k-chunks on axis 1
    #  w1/w2/w3 packed k-chunk-major: [128, KD*H] etc., bf16
    xt = nc.dram_tensor("xt", [P, KD, cap], BF16, kind="ExternalInput")
    w1d = nc.dram_tensor("w1p", [P, NW1], BF16, kind="ExternalInput")
    w2d = nc.dram_tensor("w2p", [P, NW2], BF16, kind="ExternalInput")
    w3d = nc.dram_tensor("w3p", [P, NW3], BF16, kind="ExternalInput")
    if b_width:
        w1bd = nc.dram_tensor("w1pB", [P, NW1], BF16, kind="ExternalInput")
        w2bd = nc.dram_tensor("w2pB", [P, NW2], BF16, kind="ExternalInput")
        w3bd = nc.dram_tensor("w3pB", [P, NW3], BF16, kind="ExternalInput")
    if with_bias:
        # biases as single-partition rows, pre-cast to bf16 on host:
        # cols [0,H) = b1, [H,H+H2) = b2, [H+H2,H+H2+O) = b3
        bias = nc.dram_tensor("bias", [1, H + H2 + O], BF16, kind="ExternalInput")
        if b_width:
            biasb = nc.dram_tensor(
                "biasB", [1, H + H2 + O], BF16, kind="ExternalInput")
    out = nc.dram_tensor("out", [O, cap], F32, kind="ExternalOutput")

    relu_kw = dict(op0=mybir.AluOpType.max)
    all_tws = list(tws) + ([b_width] if b_width else [])

    with tile.TileContext(nc) as tc:
        with (
            tc.tile_pool(name="weights", bufs=1) as wpool,
            tc.tile_pool(name="xin", bufs=1) as xpool,
            tc.tile_pool(name="ps1", bufs=3, space="PSUM") as ps1pool,
            tc.tile_pool(name="ps2", bufs=3, space="PSUM") as ps2pool,
            tc.tile_pool(name="ps3", bufs=2, space="PSUM") as ps3pool,
            tc.tile_pool(name="acts", bufs=2) as apool,
        ):
            # PE warm-up: memset a dummy weight + stream tile, then issue
            # matmuls that depend on no DMA. They run during the input-DMA
            # window, flipping the HAM clock gate to full rate before real
            # compute starts.
            wdum = wpool.tile([P, P], BF16, name="wdum")
            nc.vector.memset(wdum, 0.0)
            xdum = wpool.tile([P, 512], BF16, name="xdum")
            nc.vector.memset(xdum, 0.0)
            ps_warm = ps1pool.tile([P, 512], F32, tag="ps1", name="warm")
            for _ in range(N_WARM):
                nc.tensor.matmul(ps_warm, wdum, xdum, start=True, stop=True)

            # Input DMAs, all on the sync HWDGE ring, ordered so the bytes
            # that gate the earliest compute arrive first. The ring is FIFO
            # per engine, so issue order == arrival order.
            MGOFF = np.cumsum([0] + W1GROUPS)
            w1g_tiles = []

            def w1_group_dma(g, dram, via=nc.sync):
                m0, m1 = MGOFF[g], MGOFF[g + 1]
                w1g = wpool.tile(
                    [P, (m1 - m0) * KD * P], BF16, tag=f"w1g{g}")
                via.dma_start(w1g, dram[:, m0 * KD * P:m1 * KD * P])
                return w1g

            xsb_tiles = []
            xoffs = np.cumsum([0] + all_tws)

            # Everything rides the single sync HWDGE ring: FIFO order is
            # priority order, and the critical transfers get full
            # bandwidth instead of a round-robin share. x tile 0 is split
            # in half-k pieces so the first matmuls gate on 0.5 MB less.
            def x_tile_dma(t, split=False):
                xsb = xpool.tile([P, KD, all_tws[t]], BF16, tag=f"x{t}")
                if split:
                    nc.sync.dma_start(
                        xsb[:, :KD // 2, :],
                        xt[:, :KD // 2, xoffs[t]:xoffs[t + 1]])
                    nc.sync.dma_start(
                        xsb[:, KD // 2:, :],
                        xt[:, KD // 2:, xoffs[t]:xoffs[t + 1]])
                else:
                    nc.sync.dma_start(xsb, xt[:, :, xoffs[t]:xoffs[t + 1]])
                xsb_tiles.append(xsb)

            w1g_tiles.append(w1_group_dma(0, w1d))
            x_tile_dma(0, split=True)
            for g in range(1, len(W1GROUPS)):
                w1g_tiles.append(w1_group_dma(g, w1d))
            if len(all_tws) > 1:
                x_tile_dma(1)
            w2sb = wpool.tile([P, NW2], BF16, tag="w2")
            nc.sync.dma_start(w2sb[:, :NW2 // 2], w2d[:, :NW2 // 2])
            nc.sync.dma_start(w2sb[:, NW2 // 2:], w2d[:, NW2 // 2:])
            for t in range(2, len(all_tws)):
                x_tile_dma(t)
            w3sb = wpool.tile([P, NW3], BF16)
            nc.sync.dma_start(w3sb, w3d[:, :])
            if b_width:
                # W1B/W3B into their own SBUF space: no hazard, load early
                # on the HWDGE ring behind everything else. (W2B instead
                # reuses W2A's space mid-kernel, below.)
                w1sb_b = wpool.tile([P, NW1], BF16, name="w1B")
                nc.sync.dma_start(w1sb_b, w1bd[:, :])
                w3sb_b = wpool.tile([P, NW3], BF16, name="w3B")
                nc.sync.dma_start(w3sb_b, w3bd[:, :])

            def w1s_grouped(tiles):
                def f(k, m):
                    g = int(np.searchsorted(MGOFF, m, side="right")) - 1
                    off = ((m - MGOFF[g]) * KD + k) * P
                    return tiles[g][:, off:off + P]
                return f

            def w1s_flat(sb):
                def f(k, m):
                    off = (m * KD + k) * P
                    return sb[:, off:off + P]
                return f

            def w2s(sb, k, m):
                off = (m * MH + k) * P
                return sb[:, off:off + P]

            def w3s(sb, k):
                off = k * O
                return sb[:, off:off + O]

            if with_bias:
                # Bias folded into each accumulation group as one extra K=1
                # matmul against a ones row: psum[m, n] += b[m] * 1. This
                # keeps bias handling entirely on the PE, so no evacuation
                # instruction ever needs a second semaphore wait.
                bsb = wpool.tile([1, H + H2 + O], BF16, name="biasA")
                nc.sync.dma_start(bsb, bias[:, :])
                if b_width:
                    bsb_b = wpool.tile([1, H + H2 + O], BF16, name="biasB")
                    nc.sync.dma_start(bsb_b, biasb[:, :])
                ones = wpool.tile([1, max(all_tws)], BF16)
                nc.vector.memset(ones, 1.0)

            def bias_mm(bt, ps, lo, hi, tw):
                if with_bias:
                    nc.tensor.matmul(
                        ps, bt[:, lo:hi], ones[:, :tw], start=False, stop=True
                    )

            # 1-element DVE reads of the previous tile's activation buffers.
            # Slot reuse makes the first evacuation of a tile WAW-depend on
            # the previous tile's writes; the fence absorbs that own-engine
            # completion wait so no evacuation needs two semaphore waits
            # (the ISA wait slot fits only one).
            fence = wpool.tile([1, 4], BF16)
            prev = {}

            def dve_fence(key, ap):
                if key in prev:
                    nc.vector.tensor_copy(fence[:, 0:1], prev[key])
                prev[key] = ap

            # One output staging buffer for all tiles: DVE copies land in
            # disjoint column slices, and because DVE executes in FIFO
            # order a single wait on its last copy covers every slice -- so
            # the one final output DMA needs exactly one sync wait (this
            # walrus rejects instructions with more).
            osb = wpool.tile([O, cap], F32, name="osb")

            def mlp_tile(t, tw, w1t, w2t, w3t, bt, strips=True):
                tok = slice(int(xoffs[t]), int(xoffs[t]) + tw)
                xsb = xsb_tiles[t]

                h1sb = apool.tile([P, MH, tw], BF16, tag="h1")
                dve_fence("h1", h1sb[0:1, 0, 0:1])
                for m in range(MH):
                    ps = ps1pool.tile(
                        [P, 512], F32, tag="ps1", name="ps1t")[:, :tw]
                    for k in range(KD):
                        nc.tensor.matmul(
                            ps,
                            w1t(k, m),
                            xsb[:, k, :],
                            start=(k == 0),
                            stop=(k == KD - 1) and not with_bias,
                        )
                    bias_mm(bt, ps, m * P, (m + 1) * P, tw)
                    nc.vector.tensor_scalar(
                        h1sb[:, m, :], ps, 0.0, None, **relu_kw
                    )

                h2sb = apool.tile([P, MH2, tw], BF16, tag="h2")
                dve_fence("h2", h2sb[0:1, 0, 0:1])
                for m in range(MH2):
                    ps = ps2pool.tile(
                        [P, 512], F32, tag="ps2", name="ps2t")[:, :tw]
                    for k in range(MH):
                        nc.tensor.matmul(
                            ps,
                            w2s(w2t, k, m),
                            h1sb[:, k, :],
                            start=(k == 0),
                            stop=(k == MH - 1) and not with_bias,
                        )
                    bias_mm(bt, ps, H + m * P, H + (m + 1) * P, tw)
                    nc.vector.tensor_scalar(
                        h2sb[:, m, :], ps, 0.0, None, **relu_kw
                    )

                # Layer 3, column-group packed: the 8 k-chunks run
                # 4-at-a-time in separate 32-column groups of the PE array,
                # each group accumulating its own 10-row strip at PSUM
                # partition 32j. The strips are then reduced on the DVE.
                ps3 = ps3pool.tile([P, 512], F32, tag="ps3", name="ps3t")[:, :tw]
                if strips:
                    for r in range(2):
                        for j in range(NSTRIP):
                            k = r * NSTRIP + j
                            nc.tensor.matmul(
                                ps3[32 * j:32 * j + O, :],
                                w3s(w3t, k),
                                h2sb[:, k, :],
                                start=(r == 0),
                                stop=(r == 1) and not with_bias,
                                tile_position=(0, 32 * j),
                                skip_group_check=True,
                            )
                else:
                    # plain 8-matmul layer 3: slightly more PE time, but a
                    # single-copy evacuation - used for the final tile so
                    # the tail dependency chain is as short as possible
                    for k in range(MH2):
                        nc.tensor.matmul(
                            ps3[0:O, :], w3s(w3t, k), h2sb[:, k, :],
                            start=(k == 0),
                            stop=(k == MH2 - 1) and not with_bias,
                        )
                if with_bias:
                    # bias lands once, in strip 0's accumulation
                    nc.tensor.matmul(
                        ps3[0:O, :], bt[:, H + H2:H + H2 + O],
                        ones[:, :tw], start=False, stop=True,
                        tile_position=(0, 0), skip_group_check=True,
                    )
                # Reduce the strips into the staging buffer.
                nc.vector.tensor_copy(osb[:, tok], ps3[0:O, :])
                if strips:
                    for j in range(1, NSTRIP):
                        nc.vector.tensor_add(
                            osb[:, tok], ps3[32 * j:32 * j + O, :], osb[:, tok]
                        )

            for t, tw in enumerate(tws):
                mlp_tile(t, tw, w1s_grouped(w1g_tiles), w2sb, w3sb,
                         bsb if with_bias else None)
            if b_width:
                # W2B reuses W2A's SBUF. Two 1-element gpsimd fence reads
                # of the A instance absorb the reload's write-after-write
                # waits (one per A-half HWDGE lane); being Q7 compute ops,
                # program order alone sequences them before the reload
                # DMAs, which then carry only their write-after-read wait
                # on the last A-tile layer-2 matmuls of the corresponding
                # m-groups. m-major packing means half 1 frees as soon as
                # m-group 3 is done.
                gscr = wpool.tile([1, 4], BF16, name="gscr")
                nc.gpsimd.tensor_copy(gscr[0:1, 0:1], w2sb[0:1, 0:1])
                nc.gpsimd.tensor_copy(
                    gscr[0:1, 1:2], w2sb[0:1, NW2 // 2:NW2 // 2 + 1])
                # third fence absorbs the fences' own-engine completion
                # wait so the reload DMAs keep a single (PE) wait each
                nc.gpsimd.tensor_copy(gscr[0:1, 2:4], gscr[0:1, 0:2])
                w2sb_b = wpool.tile([P, NW2], BF16, tag="w2")
                # quarter-DMAs chase the A-expert's m-pair consumption so
                # each piece starts as early as its region frees up
                for q in range(4):
                    nc.gpsimd.dma_start(
                        w2sb_b[:, q * NW2 // 4:(q + 1) * NW2 // 4],
                        w2bd[:, q * NW2 // 4:(q + 1) * NW2 // 4])
                # A-part output ships while the B tile computes; the final
                # drain then only waits on the B part's 5 KB transfer.
                nc.gpsimd.dma_start(out[:, :A_CAP], osb[:, :A_CAP])
                mlp_tile(len(tws), b_width, w1s_flat(w1sb_b), w2sb_b,
                         w3sb_b, bsb_b if with_bias else None,
                         strips=False)
                nc.gpsimd.dma_start(out[:, A_CAP:], osb[:, A_CAP:])
            else:
                nc.gpsimd.dma_start(out[:, :], osb)
    return nc


_NC_CACHE: dict = {}


def _get_nc(with_bias: bool, tws, b_width=None) -> bass.Bass:
    key = (with_bias, tuple(tws), b_width)
    if key not in _NC_CACHE:
        _NC_CACHE[key] = _build_nc(with_bias, tws, b_width)
    return _NC_CACHE[key]


def _route(x, Wr, br):
    """Host router: softmax over logits, top-2, renormalized weights."""
    logits = x.astype(np.float32) @ Wr.astype(np.float32) + br.astype(np.float32)
    m = logits.max(axis=-1, keepdims=True)
    p = np.exp(logits - m)
    p /= p.sum(axis=-1, keepdims=True)
    top_i = np.argsort(-p, axis=-1, kind="stable")[:, :TOP_K]
    top_p = np.take_along_axis(p, top_i, axis=-1)
    top_p = top_p / top_p.sum(axis=-1, keepdims=True)
    return top_i.astype(np.int64), top_p.astype(np.float32)


def _pack_x(x, tok, cap):
    """Gather+transpose tokens into the [P, KD, cap] on-chip layout."""
    xt = np.zeros((P, KD, cap), _nbf16)
    if len(tok):
        xg = x[tok].astype(_nbf16).T.reshape(KD, P, len(tok))
        xt[:, :, :len(tok)] = xg.transpose(1, 0, 2)
    return xt


def _balance(tok_by_e):
    """Assign tokens to the static slot structure.

    Returns (segments, leftover) where segments is a list of
    (core, expert, tok_slice_into_tok_by_e, slot_offset) and leftover[e]
    is the index where unplaced tokens of expert e start (processed by
    overflow rounds).
    """
    segments = []
    b_used = [False] * NCORES
    b_exp = [None] * NCORES
    pos = [0] * NCORES  # consumed per expert
    spills = []
    for e in range(NCORES):
        n = len(tok_by_e[e])
        na = min(n, A_CAP)
        if na:
            segments.append((e, e, (0, na), 0))
        pos[e] = na
        rem = n - na
        if rem > 0:
            nb = min(rem, B_W)
            segments.append((e, e, (na, na + nb), A_CAP))
            b_used[e] = True
            b_exp[e] = e
            pos[e] = na + nb
            rem -= nb
            while rem > 0:
                nb = min(rem, B_W)
                spills.append((e, pos[e], pos[e] + nb))
                pos[e] += nb
                rem -= nb
    failed = set()
    for e, lo, hi in spills:
        if e in failed:
            continue
        free = [c for c in range(NCORES) if not b_used[c]]
        if not free:
            # back out: overflow rounds handle everything from lo on
            pos[e] = lo
            failed.add(e)
            continue
        c = free[0]
        segments.append((c, e, (lo, hi), A_CAP))
        b_used[c] = True
        b_exp[c] = e
    for c in range(NCORES):
        if b_exp[c] is None:
            b_exp[c] = c  # dummy weights, zero tokens
    return segments, b_exp, pos


def _run_rounds(x, top_i, top_p, W1, b1, W2, b2, W3, b3, trace=False):
    """Dispatch tokens to cores, run the NEFF(s), combine."""
    with_bias = bool(np.any(b1) or np.any(b2) or np.any(b3))

    # Static per-expert weight inputs, packed into the on-chip layout:
    # [128 partitions, k-chunk-major columns] per weight matrix.
    w1p, w2p, w3p, biasp = [], [], [], []
    for e in range(NCORES):
        # w1 m-major: [p, m, k, c] so the first m-groups lead the DMA
        w1p.append(np.ascontiguousarray(
            W1[e].reshape(KD, P, MH, P).transpose(1, 2, 0, 3).reshape(P, NW1)
        ).astype(_nbf16))
        # w2 m-major: [p, m, k, c] so the B-expert reload can chase the
        # A-expert consumption m-group by m-group
        w2p.append(np.ascontiguousarray(
            W2[e].reshape(MH, P, MH2, P).transpose(1, 2, 0, 3).reshape(P, NW2)
        ).astype(_nbf16))
        w3p.append(np.ascontiguousarray(
            W3[e].reshape(MH2, P, O).transpose(1, 0, 2).reshape(P, NW3)
        ).astype(_nbf16))
        if with_bias:
            biasp.append(np.concatenate(
                [b1[e], b2[e], b3[e]]
            ).reshape(1, H + H2 + O).astype(_nbf16))

    # (token, slot) pairs per expert.
    tok_by_e = []
    wt_by_e = []
    for e in range(NCORES):
        tok, slot = np.nonzero(top_i == e)
        tok_by_e.append(tok)
        wt_by_e.append(top_p[tok, slot])

    out = np.zeros((B, O), np.float32)

    # ---- Round 1: the balanced full NEFF ----
    segments, b_exp, offset = _balance(tok_by_e)
    nc = _get_nc(with_bias, A_TWS, B_W)
    in_maps = []
    for c in range(NCORES):
        m = {"w1p": w1p[c], "w2p": w2p[c], "w3p": w3p[c],
             "w1pB": w1p[b_exp[c]], "w2pB": w2p[b_exp[c]],
             "w3pB": w3p[b_exp[c]]}
        if with_bias:
            m["bias"] = biasp[c]
            m["biasB"] = biasp[b_exp[c]]
        xt = np.zeros((P, KD, C), _nbf16)
        for cc, e, (lo, hi), soff in segments:
            if cc != c:
                continue
            tokens = tok_by_e[e][lo:hi]
            xg = x[tokens].astype(_nbf16).T.reshape(KD, P, len(tokens))
            xt[:, :, soff:soff + len(tokens)] = xg.transpose(1, 0, 2)
        m["xt"] = np.ascontiguousarray(xt)
        in_maps.append(m)
    res = run_bass_kernel_spmd(
        nc, in_maps, core_ids=list(range(NCORES)), trace=trace
    )
    last_result = res
    for c, e, (lo, hi), soff in segments:
        tokens = tok_by_e[e][lo:hi]
        y = res.results[c]["out"][:, soff:soff + len(tokens)].T  # [n, O]
        w = wt_by_e[e][lo:hi]
        np.add.at(out, tokens, w[:, None] * y)

    # ---- Overflow rounds (generic correctness; unused for the benchmark
    # routing): any tokens _balance couldn't place run through a small
    # single-tile NEFF on their expert's core. ----
    while any(offset[e] < len(tok_by_e[e]) for e in range(NCORES)):
        cap = sum(OVERFLOW_TWS)
        nc = _get_nc(with_bias, OVERFLOW_TWS, None)
        in_maps = []
        chunks = []
        for e in range(NCORES):
            tok = tok_by_e[e][offset[e]:offset[e] + cap]
            chunks.append(tok)
            m = {"xt": np.ascontiguousarray(_pack_x(x, tok, cap)),
                 "w1p": w1p[e], "w2p": w2p[e], "w3p": w3p[e]}
            if with_bias:
                m["bias"] = biasp[e]
            in_maps.append(m)
        res = run_bass_kernel_spmd(
            nc, in_maps, core_ids=list(range(NCORES)), trace=trace
        )
        last_result = res
        for e in range(NCORES):
            tok = chunks[e]
            if len(tok) == 0:
                continue
            y = res.results[e]["out"][:, :len(tok)].T
            w = wt_by_e[e][offset[e]:offset[e] + len(tok)]
            np.add.at(out, tok, w[:, None] * y)
            offset[e] += len(tok)
    return out, last_result


def kernel(x, Wr, br, W1, b1, W2, b2, W3, b3):
    x = np.asarray(x, np.float32)
    top_i, top_p = _route(x, np.asarray(Wr), np.asarray(br))
    out, _ = _run_rounds(
        x, top_i, top_p,
        np.asarray(W1), np.asarray(b1), np.asarray(W2), np.asarray(b2),
        np.asarray(W3), np.asarray(b3),
    )
    return out


def run_traced(x, Wr, br, W1, b1, W2, b2, W3, b3):
    """Like kernel() but returns (out, BassKernelResults) with profile info."""
    x = np.asarray(x, np.float32)
    top_i, top_p = _route(x, np.asarray(Wr), np.asarray(br))
    return _run_rounds(
        x, top_i, top_p,
        np.asarray(W1), np.asarray(b1), np.asarray(W2), np.asarray(b2),
        np.asarray(W3), np.asarray(b3),
        trace=True,
    )
